# revision 1
# baseline (speedup 1.0000x reference)
"""Trainium2 Bass kernel for the batch ConsistencyLoss (masked pairwise KL).

Math (reference):
    emb = x / ||x||;  sim = emb @ emb.T;  mask = (sim > 0.8) & ~eye
    L = log_softmax(routing);  P = exp(L);  ne[j] = sum_k P[j,k] L[j,k]
    kl[i,j] = ne[j] - (L @ P.T)[i,j]
    loss = sum(mask * kl) / count(mask)

Device algorithm (per core, row strip S of 1024 rows):
  * Embeddings are transposed RAW (bf16) — no per-row normalization pass.
    Row norms come from one squared-column reduction via a ones-matmul; the
    similarity test is normalized on the threshold side instead:
        r_ij = x_i . x_j   (raw bf16 matmul)
        mask = (r_ij * (1/n_i)) > (0.8 * n_j)
    with 1/n_i per-partition ([128,1] per row chunk) and 0.8*n_j broadcast
    into a [128, B] bf16 operand — a single fused scalar_tensor_tensor per
    PSUM tile.
  * Masked-KL sum factorization:
        sum_{i in S, j} mask[i,j]*kl[i,j]
          = sum_j ne[j]*colcount_S[j] - sum_{j,k} P[j,k]*(mask_S^T @ L_S)[j,k]
    One PSUM-accumulated matmul U = [L_S|1]^T @ mask gives both terms
    (colcount in row 16).  Diagonal pairs have kl == 0 exactly, so they stay
    in the mask and the host subtracts B from the pair count.
  * Final: masked_sum = sum(W17 ⊙ U^T) with W17 = [-P | ne], one multiply +
    one reduction; per-core (sum, count) pair summed on the host.
"""

import numpy as np

import concourse.bacc as bacc
import concourse.tile as tile
from concourse import mybir
from concourse.bass_utils import run_bass_kernel_spmd
from concourse.masks import make_identity

B, E, H = 8192, 16, 1024
NCORES = 8
STRIP = B // NCORES  # 1024 rows per core
MT = STRIP // 128    # 8 row chunks per strip
KT = H // 128        # 8 contraction tiles
NT = B // 512        # 16 column tiles of 512
BT = B // 128        # 64 batch tiles
SIM_THRESHOLD = 0.8
WEIGHT = 1.0
F32 = mybir.dt.float32
BF16 = mybir.dt.bfloat16
AX = mybir.AxisListType.X
AXY = mybir.AxisListType.XY
OP = mybir.AluOpType
AF = mybir.ActivationFunctionType


def _softmax_stats(nc, pool, x, negP_out=None, ne_out=None, L_out=None):
    """From logits tile x [128, E]: optionally write -P (f32), ne (f32
    [128,1], ne = sum_k P log P) and L (any dtype) tiles."""
    negmax = pool.tile([128, 1], F32, tag="negmax")
    nc.vector.reduce_max(out=negmax, in_=x, axis=AX, negate=True)
    e = pool.tile([128, E], F32, tag="e")
    s = pool.tile([128, 1], F32, tag="s")
    nc.scalar.activation(out=e, in_=x, func=AF.Exp, bias=negmax, scale=1.0,
                         accum_out=s)
    logs = pool.tile([128, 1], F32, tag="logs")
    nc.scalar.activation(out=logs, in_=s, func=AF.Ln)
    if L_out is not None:
        # L = (x + negmax) - log(sum)
        nc.vector.tensor_scalar(L_out, x, negmax, logs, op0=OP.add,
                                op1=OP.subtract)
    if negP_out is not None:
        rs = pool.tile([128, 1], F32, tag="rs")
        nc.vector.reciprocal(out=rs, in_=s)
        nc.vector.tensor_scalar(negP_out, e, rs, -1.0, op0=OP.mult,
                                op1=OP.mult)
    if ne_out is not None:
        L = pool.tile([128, E], F32, tag="Lf")
        nc.vector.tensor_scalar(L, x, negmax, logs, op0=OP.add, op1=OP.subtract)
        scr = pool.tile([128, E], F32, tag="nescr")
        nc.vector.tensor_tensor(out=scr, in0=negP_out, in1=L, op=OP.mult)
        # scr = -P*L; negate the reduction to get ne = +sum P*L
        nc.vector.reduce_sum(out=ne_out, in_=scr, axis=AX, negate=True)


def _kernel(tc, emb, emb_s, rp, rp_s, out_dram, reps=1, loop_iters=None):
    nc = tc.nc
    with tc.tile_pool(name="persist", bufs=1) as persist:
        embt = persist.tile([128, KT, B], BF16)        # raw x^T [h%128,kt,b]
        stript = persist.tile([128, KT, STRIP], BF16)  # raw strip columns
        nb08 = persist.tile([128, B], BF16)            # 0.8*||x_j|| bcast
        rn_strip = persist.tile([128, MT], F32)        # 1/||x_i|| strip rows
        # W17[:, bt, 0:E] = -P, W17[:, bt, E] = ne — matches Ut_all layout so
        # the final masked-sum is one elementwise mult + one reduction.
        W17 = persist.tile([128, BT, E + 1], F32)
        Lpad = persist.tile([128, MT, E + 1], BF16)
        Ut_all = persist.tile([128, BT, E + 1], F32)
        identf = persist.tile([128, 128], F32)
        identb = persist.tile([128, 128], BF16)
        ones = persist.tile([128, 1], F32)
        ones_b1 = persist.tile([128, 1], BF16)
        ones_row = persist.tile([1, 128], BF16)
        make_identity(nc, identf)
        make_identity(nc, identb)
        nc.vector.memset(ones, 1.0)
        nc.vector.memset(ones_b1, 1.0)
        nc.vector.memset(ones_row, 1.0)
        nc.vector.memset(Lpad[:, :, E], 1.0)

        args = (tc, nc, emb, emb_s, rp, rp_s, out_dram, embt, stript, nb08,
                rn_strip, W17, Lpad, Ut_all, identf, identb, ones, ones_b1,
                ones_row)
        if loop_iters is not None:
            with tc.For_i(0, loop_iters, 1):
                _phases(*args, "")
            return
        for rep in range(reps):
            _phases(*args, f"r{rep}_" if reps > 1 else "")


def _phases(tc, nc, emb, emb_s, rp, rp_s, out_dram, embt, stript, nb08,
            rn_strip, W17, Lpad, Ut_all, identf, identb, ones, ones_b1,
            ones_row, r):
    # ---- Phase A: softmax stats (full batch -P/ne; strip Lpad) ----
    # All Exp ops batch under one ACT table; the 72 Ln calls collapse into
    # ONE Ln over the collected sums (ACT table loads: ~2 instead of ~99).
    TT = BT + MT
    with tc.tile_pool(name=f"{r}smx", bufs=1) as smx:
        rp_sb = smx.tile([128, BT, E], F32, tag="rp_sb")
        rps_sb = smx.tile([128, MT, E], F32, tag="rps_sb")
        nc.sync.dma_start(
            out=rp_sb, in_=rp.rearrange("(bt p) e -> p bt e", p=128))
        nc.sync.dma_start(
            out=rps_sb, in_=rp_s.rearrange("(mt p) e -> p mt e", p=128))
        e_all = smx.tile([128, TT, E], F32, tag="e_all")
        s_all = smx.tile([128, TT], F32, tag="s_all")
        nm_all = smx.tile([128, TT], F32, tag="nm_all")
        logs_all = smx.tile([128, TT], F32, tag="logs_all")
        rs_all = smx.tile([128, TT], F32, tag="rs_all")

        def logits(t):
            return rp_sb[:, t, :] if t < BT else rps_sb[:, t - BT, :]

        for t in range(TT):
            nc.vector.reduce_max(out=nm_all[:, t:t + 1], in_=logits(t),
                                 axis=AX, negate=True)
            nc.scalar.activation(out=e_all[:, t, :], in_=logits(t),
                                 func=AF.Exp, bias=nm_all[:, t:t + 1],
                                 scale=1.0, accum_out=s_all[:, t:t + 1])
        nc.scalar.activation(out=logs_all, in_=s_all, func=AF.Ln)
        nc.vector.reciprocal(out=rs_all, in_=s_all)
        for bt in range(BT):
            nc.vector.tensor_scalar(W17[:, bt, 0:E], e_all[:, bt, :],
                                    rs_all[:, bt:bt + 1], -1.0,
                                    op0=OP.mult, op1=OP.mult)
            L = smx.tile([128, E], F32, tag="Lf", bufs=2)
            nc.vector.tensor_scalar(L, rp_sb[:, bt, :], nm_all[:, bt:bt + 1],
                                    logs_all[:, bt:bt + 1], op0=OP.add,
                                    op1=OP.subtract)
            scr = smx.tile([128, E], F32, tag="nescr", bufs=2)
            nc.vector.tensor_tensor(out=scr, in0=W17[:, bt, 0:E], in1=L,
                                    op=OP.mult)
            nc.vector.reduce_sum(out=W17[:, bt, E:E + 1], in_=scr, axis=AX,
                                 negate=True)
        for ms in range(MT):
            t = BT + ms
            nc.vector.tensor_scalar(Lpad[:, ms, 0:E], rps_sb[:, ms, :],
                                    nm_all[:, t:t + 1], logs_all[:, t:t + 1],
                                    op0=OP.add, op1=OP.subtract)

    # ---- Phase B: raw transpose of embeddings (bf16) ----
    with tc.tile_pool(name=f"{r}embp", bufs=3) as ep, \
         tc.tile_pool(name=f"{r}trps", bufs=2, space="PSUM") as trps:

        def prep(src_ap, dst_tile, nb):
            for bt in range(nb):
                x = ep.tile([128, H], F32, tag="ex")
                nc.sync.dma_start(out=x,
                                  in_=src_ap[bt * 128:(bt + 1) * 128, :])
                xb = ep.tile([128, H], BF16, tag="exb", bufs=2)
                nc.scalar.copy(out=xb, in_=x)
                tp = trps.tile([128, H], BF16, tag="tr")
                for kt in range(KT):
                    nc.tensor.transpose(tp[:, kt * 128:(kt + 1) * 128],
                                        xb[:, kt * 128:(kt + 1) * 128],
                                        identb)
                nc.vector.tensor_copy(
                    out=dst_tile[:, :, bt * 128:(bt + 1) * 128],
                    in_=tp.rearrange("p (k c) -> p k c", k=KT))

        prep(emb, embt, BT)
        prep(emb_s, stript, MT)

    # ---- Phase B2: row norms via ones-matmul; nb08 + rn_strip ----
    with tc.tile_pool(name=f"{r}nrm", bufs=2) as nrm, \
         tc.tile_pool(name=f"{r}nps", bufs=2, space="PSUM") as nps, \
         tc.tile_pool(name=f"{r}bps", bufs=2, space="PSUM") as bpsp:
        # full batch: ss[j] = sum_h x[j,h]^2, nb08 = 0.8*sqrt(ss) broadcast
        for n in range(NT):
            ssp = nps.tile([1, 512], F32, tag="ssp")
            for kt in range(KT):
                sq = nrm.tile([128, 512], BF16, tag="sq")
                src = embt[:, kt, n * 512:(n + 1) * 512]
                nc.vector.tensor_tensor(out=sq, in0=src, in1=src, op=OP.mult)
                nc.tensor.matmul(out=ssp, lhsT=ones_b1, rhs=sq,
                                 start=(kt == 0), stop=(kt == KT - 1))
            n08c = nrm.tile([1, 512], BF16, tag="n08c")
            # 0.8*sqrt(ss) = sqrt(0.64*ss)
            nc.scalar.activation(out=n08c, in_=ssp, func=AF.Sqrt, bias=0.0,
                                 scale=SIM_THRESHOLD * SIM_THRESHOLD)
            bps = bpsp.tile([128, 512], F32, tag="bps")
            nc.tensor.matmul(out=bps, lhsT=ones_row, rhs=n08c, start=True,
                             stop=True)
            nc.scalar.copy(out=nb08[:, n * 512:(n + 1) * 512], in_=bps)
        # strip rows: rn_strip[p, m] = 1/||x_{m*128+p}||
        for sn in range(STRIP // 512):
            ssp = nps.tile([1, 512], F32, tag="ssp")
            for kt in range(KT):
                sq = nrm.tile([128, 512], BF16, tag="sq")
                src = stript[:, kt, sn * 512:(sn + 1) * 512]
                nc.vector.tensor_tensor(out=sq, in0=src, in1=src, op=OP.mult)
                nc.tensor.matmul(out=ssp, lhsT=ones_b1, rhs=sq,
                                 start=(kt == 0), stop=(kt == KT - 1))
            nrow = nrm.tile([1, 512], F32, tag="nrow")
            nc.scalar.activation(out=nrow, in_=ssp, func=AF.Sqrt, bias=0.0,
                                 scale=1.0)
            rrow = nrm.tile([1, 512], F32, tag="rrow")
            nc.vector.reciprocal(out=rrow, in_=nrow)
            rps = bpsp.tile([128, 4], F32, tag="rps")
            for mm in range(4):
                nc.tensor.transpose(rps[:, mm:mm + 1],
                                    rrow[:, mm * 128:(mm + 1) * 128],
                                    identf[:1, :1])
            nc.vector.tensor_copy(out=rn_strip[:, sn * 4:(sn + 1) * 4],
                                  in_=rps)

    # ---- Phase C: raw sim matmul + fused normalize-threshold + U ----
    with tc.tile_pool(name=f"{r}simps", bufs=3, space="PSUM") as sps, \
         tc.tile_pool(name=f"{r}ups", bufs=2, space="PSUM") as ups, \
         tc.tile_pool(name=f"{r}utps", bufs=2, space="PSUM") as utps, \
         tc.tile_pool(name=f"{r}mkp", bufs=3) as mkp, \
         tc.tile_pool(name=f"{r}stg", bufs=2) as stg:
        for n in range(NT):
            u = ups.tile([E + 1, 512], F32, tag="u")
            for m in range(MT):
                sim = sps.tile([128, 512], F32, tag="sim")
                for kt in range(KT):
                    nc.tensor.matmul(
                        out=sim,
                        lhsT=stript[:, kt, m * 128:(m + 1) * 128],
                        rhs=embt[:, kt, n * 512:(n + 1) * 512],
                        start=(kt == 0), stop=(kt == KT - 1))
                msk = mkp.tile([128, 512], BF16, tag="mask")
                # mask = (r * 1/n_i) > 0.8*n_j
                nc.vector.scalar_tensor_tensor(
                    out=msk, in0=sim, scalar=rn_strip[:, m:m + 1],
                    in1=nb08[:, n * 512:(n + 1) * 512],
                    op0=OP.mult, op1=OP.is_gt)
                nc.tensor.matmul(out=u, lhsT=Lpad[:, m, :], rhs=msk,
                                 start=(m == 0), stop=(m == MT - 1))
            # stage U and transpose 128-column blocks into Ut_all
            ust = stg.tile([E + 1, 512], F32, tag="ust")
            nc.scalar.copy(out=ust, in_=u)
            for c in range(4):
                jt = n * 4 + c
                tp = utps.tile([128, E + 1], F32, tag="ut")
                nc.tensor.matmul(out=tp,
                                 lhsT=ust[:, c * 128:(c + 1) * 128],
                                 rhs=identf[:E + 1, :E + 1],
                                 start=True, stop=True)
                if c % 2 == 0:
                    nc.vector.tensor_copy(out=Ut_all[:, jt, :], in_=tp)
                else:
                    nc.scalar.copy(out=Ut_all[:, jt, :], in_=tp)

    # ---- Phase D: final reduction to (masked_sum, count) ----
    with tc.tile_pool(name=f"{r}fin", bufs=1) as fin, \
         tc.tile_pool(name=f"{r}fps", bufs=1, space="PSUM") as fps:
        scr = fin.tile([128, BT, E + 1], F32)
        nc.vector.tensor_tensor(out=scr, in0=W17, in1=Ut_all, op=OP.mult)
        accs = fin.tile([128, 2], F32)
        nc.vector.reduce_sum(out=accs[:, 0:1], in_=scr, axis=AXY)
        nc.vector.reduce_sum(out=accs[:, 1:2], in_=Ut_all[:, :, E:E + 1],
                             axis=AXY)
        res = fps.tile([1, 2], F32)
        nc.tensor.matmul(out=res, lhsT=ones, rhs=accs, start=True, stop=True)
        out_sb = fin.tile([1, 2], F32)
        nc.scalar.copy(out=out_sb, in_=res)
        nc.sync.dma_start(out=out_dram, in_=out_sb)


def build_bass(reps=1, loop_iters=None):
    nc = bacc.Bacc("TRN2", target_bir_lowering=False, debug=False)
    emb = nc.dram_tensor("emb", [B, H], F32, kind="ExternalInput").ap()
    emb_s = nc.dram_tensor("emb_strip", [STRIP, H], F32,
                           kind="ExternalInput").ap()
    rp = nc.dram_tensor("rp", [B, E], F32, kind="ExternalInput").ap()
    rp_s = nc.dram_tensor("rp_strip", [STRIP, E], F32,
                          kind="ExternalInput").ap()
    out = nc.dram_tensor("out", [1, 2], F32, kind="ExternalOutput").ap()
    with tile.TileContext(nc) as tc:
        _kernel(tc, emb, emb_s, rp, rp_s, out, reps=reps,
                loop_iters=loop_iters)
    nc.compile()
    return nc


_NC_CACHE = None


def kernel(routing_probs: np.ndarray, input_embeddings: np.ndarray,
           **_unused) -> np.ndarray:
    global _NC_CACHE
    if _NC_CACHE is None:
        _NC_CACHE = build_bass()
    nc = _NC_CACHE
    rp = np.ascontiguousarray(routing_probs, dtype=np.float32)
    emb = np.ascontiguousarray(input_embeddings, dtype=np.float32)
    in_maps = []
    for d in range(NCORES):
        in_maps.append({
            "emb": emb,
            "emb_strip": np.ascontiguousarray(emb[d * STRIP:(d + 1) * STRIP]),
            "rp": rp,
            "rp_strip": np.ascontiguousarray(rp[d * STRIP:(d + 1) * STRIP]),
        })
    res = run_bass_kernel_spmd(nc, in_maps, core_ids=list(range(NCORES)))
    vals = np.array([r["out"].reshape(2) for r in res.results],
                    dtype=np.float64)
    total = vals[:, 0].sum()
    cnt = vals[:, 1].sum() - B  # drop the diagonal pairs (kl there is 0)
    if cnt > 0:
        loss = np.float32(total) / np.float32(max(cnt, 1.0))
    else:
        loss = 0.0
    return np.array(WEIGHT * loss, dtype=np.float32)



# revision 2
# speedup vs baseline: 3.7086x; 3.7086x over previous
"""Trainium2 Bass kernel for the batch ConsistencyLoss (masked pairwise KL).

Math (reference):
    emb = x / ||x||;  sim = emb @ emb.T;  mask = (sim > 0.8) & ~eye
    L = log_softmax(routing);  P = exp(L);  ne[j] = sum_k P[j,k] L[j,k]
    kl[i,j] = ne[j] - (L @ P.T)[i,j]
    loss = sum(mask * kl) / count(mask)

Device algorithm (per core, row strip S of 1024 rows):
  * Embeddings are transposed RAW (bf16) — no per-row normalization pass.
    Row norms come from one squared-column reduction via a ones-matmul; the
    similarity test is normalized on the threshold side instead:
        r_ij = x_i . x_j   (raw bf16 matmul)
        mask = (r_ij * (1/n_i)) > (0.8 * n_j)
    with 1/n_i per-partition ([128,1] per row chunk) and 0.8*n_j broadcast
    into a [128, B] bf16 operand — a single fused scalar_tensor_tensor per
    PSUM tile.
  * Masked-KL sum factorization:
        sum_{i in S, j} mask[i,j]*kl[i,j]
          = sum_j ne[j]*colcount_S[j] - sum_{j,k} P[j,k]*(mask_S^T @ L_S)[j,k]
    One PSUM-accumulated matmul U = [L_S|1]^T @ mask gives both terms
    (colcount in row 16).  Diagonal pairs have kl == 0 exactly, so they stay
    in the mask and the host subtracts B from the pair count.
  * Final: masked_sum = sum(W17 ⊙ U^T) with W17 = [-P | ne], one multiply +
    one reduction; per-core (sum, count) pair summed on the host.
"""

import numpy as np

import concourse.bacc as bacc
import concourse.tile as tile
from concourse import mybir
from concourse.bass_utils import run_bass_kernel_spmd
from concourse.masks import make_identity

B, E, H = 8192, 16, 1024
NCORES = 8
STRIP = B // NCORES  # 1024 rows per core
MT = STRIP // 128    # 8 row chunks per strip
KT = H // 128        # 8 contraction tiles
NT = B // 512        # 16 column tiles of 512
BT = B // 128        # 64 batch tiles
SIM_THRESHOLD = 0.8
WEIGHT = 1.0
F32 = mybir.dt.float32
BF16 = mybir.dt.bfloat16
AX = mybir.AxisListType.X
AXY = mybir.AxisListType.XY
OP = mybir.AluOpType
AF = mybir.ActivationFunctionType


def _softmax_stats(nc, pool, x, negP_out=None, ne_out=None, L_out=None):
    """From logits tile x [128, E]: optionally write -P (f32), ne (f32
    [128,1], ne = sum_k P log P) and L (any dtype) tiles."""
    negmax = pool.tile([128, 1], F32, tag="negmax")
    nc.vector.reduce_max(out=negmax, in_=x, axis=AX, negate=True)
    e = pool.tile([128, E], F32, tag="e")
    s = pool.tile([128, 1], F32, tag="s")
    nc.scalar.activation(out=e, in_=x, func=AF.Exp, bias=negmax, scale=1.0,
                         accum_out=s)
    logs = pool.tile([128, 1], F32, tag="logs")
    nc.scalar.activation(out=logs, in_=s, func=AF.Ln)
    if L_out is not None:
        # L = (x + negmax) - log(sum)
        nc.vector.tensor_scalar(L_out, x, negmax, logs, op0=OP.add,
                                op1=OP.subtract)
    if negP_out is not None:
        rs = pool.tile([128, 1], F32, tag="rs")
        nc.vector.reciprocal(out=rs, in_=s)
        nc.vector.tensor_scalar(negP_out, e, rs, -1.0, op0=OP.mult,
                                op1=OP.mult)
    if ne_out is not None:
        L = pool.tile([128, E], F32, tag="Lf")
        nc.vector.tensor_scalar(L, x, negmax, logs, op0=OP.add, op1=OP.subtract)
        scr = pool.tile([128, E], F32, tag="nescr")
        nc.vector.tensor_tensor(out=scr, in0=negP_out, in1=L, op=OP.mult)
        # scr = -P*L; negate the reduction to get ne = +sum P*L
        nc.vector.reduce_sum(out=ne_out, in_=scr, axis=AX, negate=True)


def _kernel(tc, emb, emb_s, rp, rp_s, out_dram, reps=1, loop_iters=None):
    nc = tc.nc
    with tc.tile_pool(name="persist", bufs=1) as persist:
        embt = persist.tile([128, KT, B], BF16)        # raw x^T [h%128,kt,b]
        stript = persist.tile([128, KT, STRIP], BF16)  # raw strip columns
        nb08 = persist.tile([128, B], BF16)            # 0.8*||x_j|| bcast
        rn_strip = persist.tile([128, MT], F32)        # 1/||x_i|| strip rows
        # W17[:, bt, 0:E] = -P, W17[:, bt, E] = ne — matches Ut_all layout so
        # the final masked-sum is one elementwise mult + one reduction.
        W17 = persist.tile([128, BT, E + 1], F32)
        Lpad = persist.tile([128, MT, E + 1], BF16)
        Ut_all = persist.tile([128, BT, E + 1], F32)
        identf = persist.tile([128, 128], F32)
        identb = persist.tile([128, 128], BF16)
        ones = persist.tile([128, 1], F32)
        ones_b1 = persist.tile([128, 1], BF16)
        ones_row = persist.tile([1, 128], BF16)
        make_identity(nc, identf)
        make_identity(nc, identb)
        nc.vector.memset(ones, 1.0)
        nc.vector.memset(ones_b1, 1.0)
        nc.vector.memset(ones_row, 1.0)
        nc.vector.memset(Lpad[:, :, E], 1.0)

        args = (tc, nc, emb, emb_s, rp, rp_s, out_dram, embt, stript, nb08,
                rn_strip, W17, Lpad, Ut_all, identf, identb, ones, ones_b1,
                ones_row)
        if loop_iters is not None:
            with tc.For_i(0, loop_iters, 1):
                _phases(*args, "")
            return
        for rep in range(reps):
            _phases(*args, f"r{rep}_" if reps > 1 else "")


def _phases(tc, nc, emb, emb_s, rp, rp_s, out_dram, embt, stript, nb08,
            rn_strip, W17, Lpad, Ut_all, identf, identb, ones, ones_b1,
            ones_row, r):
    # ---- Phase A: softmax stats (full batch -P/ne; strip Lpad) ----
    # All Exp ops batch under one ACT table; the 72 Ln calls collapse into
    # ONE Ln over the collected sums (ACT table loads: ~2 instead of ~99).
    TT = BT + MT
    with tc.tile_pool(name=f"{r}smx", bufs=1) as smx:
        rp_sb = smx.tile([128, BT, E], F32, tag="rp_sb")
        rps_sb = smx.tile([128, MT, E], F32, tag="rps_sb")
        nc.sync.dma_start(
            out=rp_sb, in_=rp.rearrange("(bt p) e -> p bt e", p=128))
        nc.sync.dma_start(
            out=rps_sb, in_=rp_s.rearrange("(mt p) e -> p mt e", p=128))
        e_all = smx.tile([128, TT, E], F32, tag="e_all")
        s_all = smx.tile([128, TT], F32, tag="s_all")
        nm_all = smx.tile([128, TT], F32, tag="nm_all")
        logs_all = smx.tile([128, TT], F32, tag="logs_all")
        rs_all = smx.tile([128, TT], F32, tag="rs_all")

        def logits(t):
            return rp_sb[:, t, :] if t < BT else rps_sb[:, t - BT, :]

        for t in range(TT):
            nc.vector.reduce_max(out=nm_all[:, t:t + 1], in_=logits(t),
                                 axis=AX, negate=True)
            nc.scalar.activation(out=e_all[:, t, :], in_=logits(t),
                                 func=AF.Exp, bias=nm_all[:, t:t + 1],
                                 scale=1.0, accum_out=s_all[:, t:t + 1])
        nc.scalar.activation(out=logs_all, in_=s_all, func=AF.Ln)
        nc.vector.reciprocal(out=rs_all, in_=s_all)
        for bt in range(BT):
            nc.vector.tensor_scalar(W17[:, bt, 0:E], e_all[:, bt, :],
                                    rs_all[:, bt:bt + 1], -1.0,
                                    op0=OP.mult, op1=OP.mult)
            L = smx.tile([128, E], F32, tag="Lf", bufs=2)
            nc.vector.tensor_scalar(L, rp_sb[:, bt, :], nm_all[:, bt:bt + 1],
                                    logs_all[:, bt:bt + 1], op0=OP.add,
                                    op1=OP.subtract)
            scr = smx.tile([128, E], F32, tag="nescr", bufs=2)
            nc.vector.tensor_tensor(out=scr, in0=W17[:, bt, 0:E], in1=L,
                                    op=OP.mult)
            nc.vector.reduce_sum(out=W17[:, bt, E:E + 1], in_=scr, axis=AX,
                                 negate=True)
        for ms in range(MT):
            t = BT + ms
            nc.vector.tensor_scalar(Lpad[:, ms, 0:E], rps_sb[:, ms, :],
                                    nm_all[:, t:t + 1], logs_all[:, t:t + 1],
                                    op0=OP.add, op1=OP.subtract)

    # ---- Phase B: raw transpose of embeddings (bf16) ----
    with tc.tile_pool(name=f"{r}embp", bufs=3) as ep, \
         tc.tile_pool(name=f"{r}trps", bufs=2, space="PSUM") as trps:

        def prep(src_ap, dst_tile, nb):
            for bt in range(nb):
                x = ep.tile([128, H], F32, tag="ex")
                nc.sync.dma_start(out=x,
                                  in_=src_ap[bt * 128:(bt + 1) * 128, :])
                xb = ep.tile([128, H], BF16, tag="exb", bufs=2)
                nc.scalar.copy(out=xb, in_=x)
                tp = trps.tile([128, H], BF16, tag="tr")
                for kt in range(KT):
                    nc.tensor.transpose(tp[:, kt * 128:(kt + 1) * 128],
                                        xb[:, kt * 128:(kt + 1) * 128],
                                        identb)
                nc.vector.tensor_copy(
                    out=dst_tile[:, :, bt * 128:(bt + 1) * 128],
                    in_=tp.rearrange("p (k c) -> p k c", k=KT))

        prep(emb, embt, BT)
        prep(emb_s, stript, MT)

    # ---- Phase B2: row norms via ones-matmul; nb08 + rn_strip ----
    with tc.tile_pool(name=f"{r}nrm", bufs=2) as nrm, \
         tc.tile_pool(name=f"{r}nps", bufs=2, space="PSUM") as nps, \
         tc.tile_pool(name=f"{r}bps", bufs=2, space="PSUM") as bpsp:
        # full batch: ss[j] = sum_h x[j,h]^2, nb08 = 0.8*sqrt(ss) broadcast
        for n in range(NT):
            ssp = nps.tile([1, 512], F32, tag="ssp")
            for kt in range(KT):
                sq = nrm.tile([128, 512], BF16, tag="sq")
                src = embt[:, kt, n * 512:(n + 1) * 512]
                nc.vector.tensor_tensor(out=sq, in0=src, in1=src, op=OP.mult)
                nc.tensor.matmul(out=ssp, lhsT=ones_b1, rhs=sq,
                                 start=(kt == 0), stop=(kt == KT - 1))
            n08c = nrm.tile([1, 512], BF16, tag="n08c")
            # 0.8*sqrt(ss) = sqrt(0.64*ss)
            nc.scalar.activation(out=n08c, in_=ssp, func=AF.Sqrt, bias=0.0,
                                 scale=SIM_THRESHOLD * SIM_THRESHOLD)
            bps = bpsp.tile([128, 512], F32, tag="bps")
            nc.tensor.matmul(out=bps, lhsT=ones_row, rhs=n08c, start=True,
                             stop=True)
            nc.scalar.copy(out=nb08[:, n * 512:(n + 1) * 512], in_=bps)
        # strip rows: rn_strip[p, m] = 1/||x_{m*128+p}||
        for sn in range(STRIP // 512):
            ssp = nps.tile([1, 512], F32, tag="ssp")
            for kt in range(KT):
                sq = nrm.tile([128, 512], BF16, tag="sq")
                src = stript[:, kt, sn * 512:(sn + 1) * 512]
                nc.vector.tensor_tensor(out=sq, in0=src, in1=src, op=OP.mult)
                nc.tensor.matmul(out=ssp, lhsT=ones_b1, rhs=sq,
                                 start=(kt == 0), stop=(kt == KT - 1))
            nrow = nrm.tile([1, 512], F32, tag="nrow")
            nc.scalar.activation(out=nrow, in_=ssp, func=AF.Sqrt, bias=0.0,
                                 scale=1.0)
            rrow = nrm.tile([1, 512], F32, tag="rrow")
            nc.vector.reciprocal(out=rrow, in_=nrow)
            rps = bpsp.tile([128, 4], F32, tag="rps")
            for mm in range(4):
                nc.tensor.transpose(rps[:, mm:mm + 1],
                                    rrow[:, mm * 128:(mm + 1) * 128],
                                    identf[:1, :1])
            nc.vector.tensor_copy(out=rn_strip[:, sn * 4:(sn + 1) * 4],
                                  in_=rps)

    # ---- Phase C: raw sim matmul + fused normalize-threshold + U ----
    with tc.tile_pool(name=f"{r}simps", bufs=3, space="PSUM") as sps, \
         tc.tile_pool(name=f"{r}ups", bufs=2, space="PSUM") as ups, \
         tc.tile_pool(name=f"{r}utps", bufs=2, space="PSUM") as utps, \
         tc.tile_pool(name=f"{r}mkp", bufs=3) as mkp, \
         tc.tile_pool(name=f"{r}stg", bufs=2) as stg:
        for n in range(NT):
            u = ups.tile([E + 1, 512], F32, tag="u")
            for m in range(MT):
                sim = sps.tile([128, 512], F32, tag="sim")
                for kt in range(KT):
                    nc.tensor.matmul(
                        out=sim,
                        lhsT=stript[:, kt, m * 128:(m + 1) * 128],
                        rhs=embt[:, kt, n * 512:(n + 1) * 512],
                        start=(kt == 0), stop=(kt == KT - 1))
                msk = mkp.tile([128, 512], BF16, tag="mask")
                # mask = (r * 1/n_i) > 0.8*n_j
                nc.vector.scalar_tensor_tensor(
                    out=msk, in0=sim, scalar=rn_strip[:, m:m + 1],
                    in1=nb08[:, n * 512:(n + 1) * 512],
                    op0=OP.mult, op1=OP.is_gt)
                nc.tensor.matmul(out=u, lhsT=Lpad[:, m, :], rhs=msk,
                                 start=(m == 0), stop=(m == MT - 1))
            # stage U and transpose 128-column blocks into Ut_all
            ust = stg.tile([E + 1, 512], F32, tag="ust")
            nc.scalar.copy(out=ust, in_=u)
            for c in range(4):
                jt = n * 4 + c
                tp = utps.tile([128, E + 1], F32, tag="ut")
                nc.tensor.matmul(out=tp,
                                 lhsT=ust[:, c * 128:(c + 1) * 128],
                                 rhs=identf[:E + 1, :E + 1],
                                 start=True, stop=True)
                if c % 2 == 0:
                    nc.vector.tensor_copy(out=Ut_all[:, jt, :], in_=tp)
                else:
                    nc.scalar.copy(out=Ut_all[:, jt, :], in_=tp)

    # ---- Phase D: final reduction to (masked_sum, count) ----
    with tc.tile_pool(name=f"{r}fin", bufs=1) as fin, \
         tc.tile_pool(name=f"{r}fps", bufs=1, space="PSUM") as fps:
        scr = fin.tile([128, BT, E + 1], F32)
        nc.vector.tensor_tensor(out=scr, in0=W17, in1=Ut_all, op=OP.mult)
        accs = fin.tile([128, 2], F32)
        nc.vector.reduce_sum(out=accs[:, 0:1], in_=scr, axis=AXY)
        nc.vector.reduce_sum(out=accs[:, 1:2], in_=Ut_all[:, :, E:E + 1],
                             axis=AXY)
        res = fps.tile([1, 2], F32)
        nc.tensor.matmul(out=res, lhsT=ones, rhs=accs, start=True, stop=True)
        out_sb = fin.tile([1, 2], F32)
        nc.scalar.copy(out=out_sb, in_=res)
        nc.sync.dma_start(out=out_dram, in_=out_sb)


def build_bass(reps=1, loop_iters=None):
    nc = bacc.Bacc("TRN2", target_bir_lowering=False, debug=False)
    emb = nc.dram_tensor("emb", [B, H], F32, kind="ExternalInput").ap()
    emb_s = nc.dram_tensor("emb_strip", [STRIP, H], F32,
                           kind="ExternalInput").ap()
    rp = nc.dram_tensor("rp", [B, E], F32, kind="ExternalInput").ap()
    rp_s = nc.dram_tensor("rp_strip", [STRIP, E], F32,
                          kind="ExternalInput").ap()
    out = nc.dram_tensor("out", [1, 2], F32, kind="ExternalOutput").ap()
    with tile.TileContext(nc) as tc:
        _kernel(tc, emb, emb_s, rp, rp_s, out, reps=reps,
                loop_iters=loop_iters)
    nc.compile()
    return nc


_NC_CACHE = None


def make_in_maps(rp, emb):
    in_maps = []
    for d in range(NCORES):
        in_maps.append({
            "emb": emb,
            "emb_strip": np.ascontiguousarray(emb[d * STRIP:(d + 1) * STRIP]),
            "rp": rp,
            "rp_strip": np.ascontiguousarray(rp[d * STRIP:(d + 1) * STRIP]),
        })
    return in_maps


def kernel(routing_probs: np.ndarray, input_embeddings: np.ndarray,
           **_unused) -> np.ndarray:
    global _NC_CACHE
    if _NC_CACHE is None:
        _NC_CACHE = build_bass()
    nc = _NC_CACHE
    rp = np.ascontiguousarray(routing_probs, dtype=np.float32)
    emb = np.ascontiguousarray(input_embeddings, dtype=np.float32)
    in_maps = make_in_maps(rp, emb)
    res = run_bass_kernel_spmd(nc, in_maps, core_ids=list(range(NCORES)))
    vals = np.array([r["out"].reshape(2) for r in res.results],
                    dtype=np.float64)
    total = vals[:, 0].sum()
    cnt = vals[:, 1].sum() - B  # drop the diagonal pairs (kl there is 0)
    if cnt > 0:
        loss = np.float32(total) / np.float32(max(cnt, 1.0))
    else:
        loss = 0.0
    return np.array(WEIGHT * loss, dtype=np.float32)



# revision 5
# speedup vs baseline: 4.8075x; 1.2963x over previous
"""Trainium2 Bass kernel for the batch ConsistencyLoss (masked pairwise KL).

Math (reference):
    emb = x / ||x||;  sim = emb @ emb.T;  mask = (sim > 0.8) & ~eye
    L = log_softmax(routing);  P = exp(L);  ne[j] = sum_k P[j,k] L[j,k]
    kl[i,j] = ne[j] - (L @ P.T)[i,j]
    loss = sum(mask * kl) / count(mask)

Device algorithm (per core, row strip S of 1024 rows):
  * Rows are normalized to ||u|| = 16 in the natural [row, H] layout
    (one ACT Square+accum pass for ||x||^2, one DVE multiply that also
    quantizes to fp8e4), then transposed via the PE.  The similarity
    test becomes a compare against the CONSTANT 0.8*16*16 = 204.8 --
    no per-row norm broadcasts, no second norm pass.
  * sim row-strips are computed with fp8 DoubleRow matmuls (contraction
    256 per instruction): r = q(u_i) . q(u_j), mask = r > 204.8.
  * Masked-KL sum factorization:
        sum_{i in S, j} mask[i,j]*kl[i,j]
          = sum_j ne[j]*colcount_S[j] - sum_{j,k} P[j,k]*(mask_S^T @ L_S)[j,k]
    One PSUM-accumulated matmul U = [L_S|1]^T @ mask gives both terms
    (colcount in row 16).  Diagonal pairs have kl == 0 exactly, so they stay
    in the mask and the host subtracts B from the pair count.
  * Final: masked_sum = sum(W17 * U^T) with W17 = [-P | ne], one multiply +
    one reduction; per-core (sum, count) pair summed on the host.
"""

import numpy as np

import concourse.bacc as bacc
import concourse.tile as tile
from concourse import mybir
from concourse.bass_utils import run_bass_kernel_spmd
from concourse.masks import make_identity

B, E, H = 8192, 16, 1024
NCORES = 8
STRIP = B // NCORES  # 1024 rows per core
MT = STRIP // 128    # 8 row chunks per strip
KT = H // 128        # 8 contraction tiles
KT2 = KT // 2        # 4 DoubleRow contraction pairs
NT = B // 512        # 16 column tiles of 512
BT = B // 128        # 64 batch tiles
SIM_THRESHOLD = 0.8
SCALE = 16.0         # rows normalized to this L2 norm before fp8 quant
THRESH = SIM_THRESHOLD * SCALE * SCALE  # 204.8 in raw-dot units
WEIGHT = 1.0
F32 = mybir.dt.float32
BF16 = mybir.dt.bfloat16
F8 = mybir.dt.float8e4
AX = mybir.AxisListType.X
AXY = mybir.AxisListType.XY
OP = mybir.AluOpType
AF = mybir.ActivationFunctionType
DR = mybir.MatmulPerfMode.DoubleRow


def _kernel(tc, emb, emb_s, rp, rp_s, out_dram, reps=1, loop_iters=None):
    nc = tc.nc
    with tc.tile_pool(name="persist", bufs=1) as persist:
        embt = persist.tile([128, KT, B], F8)          # q(u)^T [h%128,kt,b]
        stript = persist.tile([128, KT, STRIP], F8)    # strip columns
        # W17[:, bt, 0:E] = -P, W17[:, bt, E] = ne — matches Ut_all layout so
        # the final masked-sum is one elementwise mult + one reduction.
        W17 = persist.tile([128, BT, E + 1], F32)
        Lpad = persist.tile([128, MT, E + 1], BF16)
        Ut_all = persist.tile([128, BT, E + 1], F32)
        identf = persist.tile([128, 128], F32)
        identb = persist.tile([128, 128], BF16)
        ones = persist.tile([128, 1], F32)
        make_identity(nc, identf)
        make_identity(nc, identb)
        nc.vector.memset(ones, 1.0)
        nc.vector.memset(Lpad[:, :, E], 1.0)

        args = (tc, nc, emb, emb_s, rp, rp_s, out_dram, embt, stript,
                W17, Lpad, Ut_all, identf, identb, ones)
        if loop_iters is not None:
            with tc.For_i(0, loop_iters, 1):
                _phases(*args, "")
            return
        for rep in range(reps):
            _phases(*args, f"r{rep}_" if reps > 1 else "")


def _phases(tc, nc, emb, emb_s, rp, rp_s, out_dram, embt, stript,
            W17, Lpad, Ut_all, identf, identb, ones, r):
    # ---- Phase A: softmax stats (full batch -P/ne; strip Lpad) ----
    # All Exp ops batch under one ACT table; the 72 Ln calls collapse into
    # ONE Ln over the collected sums (ACT table loads: ~2 instead of ~99).
    TT = BT + MT
    with tc.tile_pool(name=f"{r}smx", bufs=1) as smx:
        rp_sb = smx.tile([128, BT, E], F32, tag="rp_sb")
        rps_sb = smx.tile([128, MT, E], F32, tag="rps_sb")
        nc.sync.dma_start(
            out=rp_sb, in_=rp.rearrange("(bt p) e -> p bt e", p=128))
        nc.sync.dma_start(
            out=rps_sb, in_=rp_s.rearrange("(mt p) e -> p mt e", p=128))
        e_all = smx.tile([128, TT, E], F32, tag="e_all")
        s_all = smx.tile([128, TT], F32, tag="s_all")
        nm_all = smx.tile([128, TT], F32, tag="nm_all")
        logs_all = smx.tile([128, TT], F32, tag="logs_all")
        rs_all = smx.tile([128, TT], F32, tag="rs_all")

        def logits(t):
            return rp_sb[:, t, :] if t < BT else rps_sb[:, t - BT, :]

        for t in range(TT):
            nc.vector.reduce_max(out=nm_all[:, t:t + 1], in_=logits(t),
                                 axis=AX, negate=True)
            nc.scalar.activation(out=e_all[:, t, :], in_=logits(t),
                                 func=AF.Exp, bias=nm_all[:, t:t + 1],
                                 scale=1.0, accum_out=s_all[:, t:t + 1])
        nc.scalar.activation(out=logs_all, in_=s_all, func=AF.Ln)
        nc.vector.reciprocal(out=rs_all, in_=s_all)
        for bt in range(BT):
            nc.vector.tensor_scalar(W17[:, bt, 0:E], e_all[:, bt, :],
                                    rs_all[:, bt:bt + 1], -1.0,
                                    op0=OP.mult, op1=OP.mult)
            L = smx.tile([128, E], F32, tag="Lf", bufs=2)
            nc.vector.tensor_scalar(L, rp_sb[:, bt, :], nm_all[:, bt:bt + 1],
                                    logs_all[:, bt:bt + 1], op0=OP.add,
                                    op1=OP.subtract)
            scr = smx.tile([128, E], F32, tag="nescr", bufs=2)
            nc.vector.tensor_tensor(out=scr, in0=W17[:, bt, 0:E], in1=L,
                                    op=OP.mult)
            nc.vector.reduce_sum(out=W17[:, bt, E:E + 1], in_=scr, axis=AX,
                                 negate=True)
        for ms in range(MT):
            t = BT + ms
            nc.vector.tensor_scalar(Lpad[:, ms, 0:E], rps_sb[:, ms, :],
                                    nm_all[:, t:t + 1], logs_all[:, t:t + 1],
                                    op0=OP.add, op1=OP.subtract)

    # ---- Phase B: normalize rows to ||u||=16, quantize fp8, transpose ----
    with tc.tile_pool(name=f"{r}embp", bufs=3) as ep, \
         tc.tile_pool(name=f"{r}trps", bufs=2, space="PSUM") as trps:

        def prep(src_ap, dst_tile, nb):
            for bt in range(nb):
                x = ep.tile([128, H], F32, tag="ex")
                nc.sync.dma_start(out=x,
                                  in_=src_ap[bt * 128:(bt + 1) * 128, :])
                scr = ep.tile([128, H], BF16, tag="sqscr", bufs=2)
                ss = ep.tile([128, 1], F32, tag="ss", bufs=2)
                nc.scalar.activation(out=scr, in_=x, func=AF.Square,
                                     accum_out=ss)
                # n16 = ||x||/16;  rs = 16/||x||
                n16 = ep.tile([128, 1], F32, tag="n16", bufs=2)
                nc.scalar.activation(out=n16, in_=ss, func=AF.Sqrt, bias=0.0,
                                     scale=1.0 / (SCALE * SCALE))
                rs = ep.tile([128, 1], F32, tag="rs", bufs=2)
                nc.vector.reciprocal(out=rs, in_=n16)
                xq = ep.tile([128, H], BF16, tag="xq", bufs=2)
                nc.vector.tensor_scalar(xq, x, rs, None, op0=OP.mult)
                tp = trps.tile([128, H], BF16, tag="tr")
                for kt in range(KT):
                    nc.tensor.transpose(tp[:, kt * 128:(kt + 1) * 128],
                                        xq[:, kt * 128:(kt + 1) * 128],
                                        identb)
                if bt % 2 == 0:
                    nc.scalar.copy(
                        out=dst_tile[:, :, bt * 128:(bt + 1) * 128],
                        in_=tp.rearrange("p (k c) -> p k c", k=KT))
                else:
                    nc.vector.tensor_copy(
                        out=dst_tile[:, :, bt * 128:(bt + 1) * 128],
                        in_=tp.rearrange("p (k c) -> p k c", k=KT))

        prep(emb, embt, BT)
        prep(emb_s, stript, MT)

    # ---- Phase C: fp8 DoubleRow sim matmul + threshold + U ----
    with tc.tile_pool(name=f"{r}simps", bufs=3, space="PSUM") as sps, \
         tc.tile_pool(name=f"{r}ups", bufs=2, space="PSUM") as ups, \
         tc.tile_pool(name=f"{r}utps", bufs=2, space="PSUM") as utps, \
         tc.tile_pool(name=f"{r}mkp", bufs=3) as mkp, \
         tc.tile_pool(name=f"{r}stg", bufs=2) as stg:
        for n in range(NT):
            u = ups.tile([E + 1, 512], F32, tag="u")
            for m in range(MT):
                sim = sps.tile([128, 512], F32, tag="sim")
                for k2 in range(KT2):
                    nc.tensor.matmul(
                        out=sim,
                        lhsT=stript[:, 2 * k2:2 * k2 + 2,
                                    m * 128:(m + 1) * 128],
                        rhs=embt[:, 2 * k2:2 * k2 + 2,
                                 n * 512:(n + 1) * 512],
                        start=(k2 == 0), stop=(k2 == KT2 - 1),
                        perf_mode=DR)
                msk = mkp.tile([128, 512], BF16, tag="mask")
                nc.vector.tensor_scalar(msk, sim, THRESH, None, op0=OP.is_gt)
                nc.tensor.matmul(out=u, lhsT=Lpad[:, m, :], rhs=msk,
                                 start=(m == 0), stop=(m == MT - 1))
            # stage U and transpose 128-column blocks into Ut_all
            ust = stg.tile([E + 1, 512], F32, tag="ust")
            nc.scalar.copy(out=ust, in_=u)
            for c in range(4):
                jt = n * 4 + c
                tp = utps.tile([128, E + 1], F32, tag="ut")
                nc.tensor.matmul(out=tp,
                                 lhsT=ust[:, c * 128:(c + 1) * 128],
                                 rhs=identf[:E + 1, :E + 1],
                                 start=True, stop=True)
                if c % 2 == 0:
                    nc.vector.tensor_copy(out=Ut_all[:, jt, :], in_=tp)
                else:
                    nc.scalar.copy(out=Ut_all[:, jt, :], in_=tp)

    # ---- Phase D: final reduction to (masked_sum, count) ----
    with tc.tile_pool(name=f"{r}fin", bufs=1) as fin, \
         tc.tile_pool(name=f"{r}fps", bufs=1, space="PSUM") as fps:
        scr = fin.tile([128, BT, E + 1], F32)
        nc.vector.tensor_tensor(out=scr, in0=W17, in1=Ut_all, op=OP.mult)
        accs = fin.tile([128, 2], F32)
        nc.vector.reduce_sum(out=accs[:, 0:1], in_=scr, axis=AXY)
        nc.vector.reduce_sum(out=accs[:, 1:2], in_=Ut_all[:, :, E:E + 1],
                             axis=AXY)
        res = fps.tile([1, 2], F32)
        nc.tensor.matmul(out=res, lhsT=ones, rhs=accs, start=True, stop=True)
        out_sb = fin.tile([1, 2], F32)
        nc.scalar.copy(out=out_sb, in_=res)
        nc.sync.dma_start(out=out_dram, in_=out_sb)


def build_bass(reps=1, loop_iters=None):
    nc = bacc.Bacc("TRN2", target_bir_lowering=False, debug=False)
    emb = nc.dram_tensor("emb", [B, H], F32, kind="ExternalInput").ap()
    emb_s = nc.dram_tensor("emb_strip", [STRIP, H], F32,
                           kind="ExternalInput").ap()
    rp = nc.dram_tensor("rp", [B, E], F32, kind="ExternalInput").ap()
    rp_s = nc.dram_tensor("rp_strip", [STRIP, E], F32,
                          kind="ExternalInput").ap()
    out = nc.dram_tensor("out", [1, 2], F32, kind="ExternalOutput").ap()
    with tile.TileContext(nc) as tc:
        _kernel(tc, emb, emb_s, rp, rp_s, out, reps=reps,
                loop_iters=loop_iters)
    nc.compile()
    return nc


_NC_CACHE = None


def make_in_maps(rp, emb):
    in_maps = []
    for d in range(NCORES):
        in_maps.append({
            "emb": emb,
            "emb_strip": np.ascontiguousarray(emb[d * STRIP:(d + 1) * STRIP]),
            "rp": rp,
            "rp_strip": np.ascontiguousarray(rp[d * STRIP:(d + 1) * STRIP]),
        })
    return in_maps


def kernel(routing_probs: np.ndarray, input_embeddings: np.ndarray,
           **_unused) -> np.ndarray:
    global _NC_CACHE
    if _NC_CACHE is None:
        _NC_CACHE = build_bass()
    nc = _NC_CACHE
    rp = np.ascontiguousarray(routing_probs, dtype=np.float32)
    emb = np.ascontiguousarray(input_embeddings, dtype=np.float32)
    in_maps = make_in_maps(rp, emb)
    res = run_bass_kernel_spmd(nc, in_maps, core_ids=list(range(NCORES)))
    vals = np.array([r["out"].reshape(2) for r in res.results],
                    dtype=np.float64)
    total = vals[:, 0].sum()
    cnt = vals[:, 1].sum() - B  # drop the diagonal pairs (kl there is 0)
    if cnt > 0:
        loss = np.float32(total) / np.float32(max(cnt, 1.0))
    else:
        loss = 0.0
    return np.array(WEIGHT * loss, dtype=np.float32)


# revision 12
# speedup vs baseline: 4.9332x; 1.0261x over previous
"""Trainium2 Bass kernel for the batch ConsistencyLoss (masked pairwise KL).

Math (reference):
    emb = x / ||x||;  sim = emb @ emb.T;  mask = (sim > 0.8) & ~eye
    L = log_softmax(routing);  P = exp(L);  ne[j] = sum_k P[j,k] L[j,k]
    kl[i,j] = ne[j] - (L @ P.T)[i,j]
    loss = sum(mask * kl) / count(mask)

Device algorithm (per core, row strip S of 1024 rows):
  * Rows are normalized to ||u|| = 16 in the natural [row, H] layout
    (one ACT Square+accum pass for ||x||^2, one DVE multiply that also
    quantizes to fp8e4), then transposed via the PE.  The similarity
    test becomes a compare against the CONSTANT 0.8*16*16 = 204.8 --
    no per-row norm broadcasts, no second norm pass.
  * sim row-strips are computed with fp8 DoubleRow matmuls (contraction
    256 per instruction): r = q(u_i) . q(u_j), mask = r > 204.8.
  * Masked-KL sum factorization:
        sum_{i in S, j} mask[i,j]*kl[i,j]
          = sum_j ne[j]*colcount_S[j] - sum_{j,k} P[j,k]*(mask_S^T @ L_S)[j,k]
    One PSUM-accumulated matmul U = [L_S|1]^T @ mask gives both terms
    (colcount in row 16).  Diagonal pairs have kl == 0 exactly, so they stay
    in the mask and the host subtracts B from the pair count.
  * Final: masked_sum = sum(W17 * U^T) with W17 = [-P | ne], one multiply +
    one reduction; per-core (sum, count) pair summed on the host.
"""

import numpy as np

import concourse.bacc as bacc
import concourse.tile as tile
from concourse import mybir
from concourse.bass_utils import run_bass_kernel_spmd
from concourse.masks import make_identity

B, E, H = 8192, 16, 1024
NCORES = 8
STRIP = B // NCORES  # 1024 rows per core
MT = STRIP // 128    # 8 row chunks per strip
KT = H // 128        # 8 contraction tiles
KT2 = KT // 2        # 4 DoubleRow contraction pairs
NT = B // 512        # 16 column tiles of 512
BT = B // 128        # 64 batch tiles
SIM_THRESHOLD = 0.8
SCALE = 16.0         # rows normalized to this L2 norm before fp8 quant
THRESH = SIM_THRESHOLD * SCALE * SCALE  # 204.8 in raw-dot units
WEIGHT = 1.0
F32 = mybir.dt.float32
BF16 = mybir.dt.bfloat16
F8 = mybir.dt.float8e4
AX = mybir.AxisListType.X
AXY = mybir.AxisListType.XY
OP = mybir.AluOpType
AF = mybir.ActivationFunctionType
DR = mybir.MatmulPerfMode.DoubleRow


def _kernel(tc, emb, emb_s, rp, rp_s, out_dram, reps=1, loop_iters=None,
            phases="ABCD"):
    nc = tc.nc
    with tc.tile_pool(name="persist", bufs=1) as persist:
        embt = persist.tile([128, KT, B], F8)          # q(u)^T [h%128,kt,b]
        stript = persist.tile([128, KT, STRIP], F8)    # strip columns
        # W17[:, bt, 0:E] = -P, W17[:, bt, E] = ne — matches Ut_all layout so
        # the final masked-sum is one elementwise mult + one reduction.
        W17 = persist.tile([128, BT, E + 1], F32)
        Lpad = persist.tile([128, MT, E + 1], BF16)
        Ut_all = persist.tile([128, BT, E + 1], F32)
        identf = persist.tile([128, 128], F32)
        identb = persist.tile([128, 128], BF16)
        ones = persist.tile([128, 1], F32)
        make_identity(nc, identf)
        make_identity(nc, identb)
        nc.vector.memset(ones, 1.0)
        nc.vector.memset(Lpad[:, :, E], 1.0)

        args = (tc, nc, emb, emb_s, rp, rp_s, out_dram, embt, stript,
                W17, Lpad, Ut_all, identf, identb, ones)
        if loop_iters is not None:
            with tc.For_i(0, loop_iters, 1):
                _phases(*args, "", phases)
            return
        for rep in range(reps):
            _phases(*args, f"r{rep}_" if reps > 1 else "", phases)


def _phases(tc, nc, emb, emb_s, rp, rp_s, out_dram, embt, stript,
            W17, Lpad, Ut_all, identf, identb, ones, r, which="ABCD"):
    # ---- Phase A: softmax stats (full batch -P/ne; strip Lpad) ----
    # All Exp ops batch under one ACT table; the 72 Ln calls collapse into
    # ONE Ln over the collected sums (ACT table loads: ~2 instead of ~99).
    TT = BT + MT
    if "A" not in which:
        pass
    else:
     with tc.tile_pool(name=f"{r}smx", bufs=1) as smx:
        rp_sb = smx.tile([128, BT, E], F32, tag="rp_sb")
        rps_sb = smx.tile([128, MT, E], F32, tag="rps_sb")
        nc.sync.dma_start(
            out=rp_sb, in_=rp.rearrange("(bt p) e -> p bt e", p=128))
        nc.sync.dma_start(
            out=rps_sb, in_=rp_s.rearrange("(mt p) e -> p mt e", p=128))
        e_all = smx.tile([128, TT, E], F32, tag="e_all")
        s_all = smx.tile([128, TT], F32, tag="s_all")
        nm_all = smx.tile([128, TT], F32, tag="nm_all")
        logs_all = smx.tile([128, TT], F32, tag="logs_all")
        rs_all = smx.tile([128, TT], F32, tag="rs_all")

        def logits(t):
            return rp_sb[:, t, :] if t < BT else rps_sb[:, t - BT, :]

        for t in range(TT):
            nc.vector.reduce_max(out=nm_all[:, t:t + 1], in_=logits(t),
                                 axis=AX, negate=True)
            nc.scalar.activation(out=e_all[:, t, :], in_=logits(t),
                                 func=AF.Exp, bias=nm_all[:, t:t + 1],
                                 scale=1.0, accum_out=s_all[:, t:t + 1])
        nc.scalar.activation(out=logs_all, in_=s_all, func=AF.Ln)
        nc.vector.reciprocal(out=rs_all, in_=s_all)
        for bt in range(BT):
            nc.vector.tensor_scalar(W17[:, bt, 0:E], e_all[:, bt, :],
                                    rs_all[:, bt:bt + 1], -1.0,
                                    op0=OP.mult, op1=OP.mult)
            L = smx.tile([128, E], F32, tag="Lf", bufs=2)
            nc.vector.tensor_scalar(L, rp_sb[:, bt, :], nm_all[:, bt:bt + 1],
                                    logs_all[:, bt:bt + 1], op0=OP.add,
                                    op1=OP.subtract)
            scr = smx.tile([128, E], F32, tag="nescr", bufs=2)
            nc.vector.tensor_tensor(out=scr, in0=W17[:, bt, 0:E], in1=L,
                                    op=OP.mult)
            nc.vector.reduce_sum(out=W17[:, bt, E:E + 1], in_=scr, axis=AX,
                                 negate=True)
        for ms in range(MT):
            t = BT + ms
            nc.vector.tensor_scalar(Lpad[:, ms, 0:E], rps_sb[:, ms, :],
                                    nm_all[:, t:t + 1], logs_all[:, t:t + 1],
                                    op0=OP.add, op1=OP.subtract)

    # ---- Phase B: normalize rows to ||u||=16, quantize fp8, transpose ----
    if "B" in which:
     with tc.tile_pool(name=f"{r}embp", bufs=3) as ep, \
         tc.tile_pool(name=f"{r}trps", bufs=3, space="PSUM") as trps:

        # Software-pipelined: the PSUM->SBUF copy-out of tile bt is emitted
        # LAG tiles later so the in-order ACT/DVE streams never block on the
        # PE transposes of the tile they just normalized.
        LAG = 2

        def prep(src_ap, dst_tile, nb, flush):
            pend = []

            def drain():
                tp_, bt_ = pend.pop(0)
                eng = (nc.scalar.copy if bt_ % 2 == 0
                       else nc.vector.tensor_copy)
                eng(out=dst_tile[:, :, bt_ * 128:(bt_ + 1) * 128],
                    in_=tp_.rearrange("p (k c) -> p k c", k=KT))

            for bt in range(nb):
                x = ep.tile([128, H], F32, tag="ex", bufs=4)
                nc.sync.dma_start(out=x,
                                  in_=src_ap[bt * 128:(bt + 1) * 128, :])
                scr = ep.tile([128, H], BF16, tag="sqscr", bufs=2)
                ss = ep.tile([128, 1], F32, tag="ss", bufs=3)
                nc.scalar.activation(out=scr, in_=x, func=AF.Square,
                                     accum_out=ss)
                # n16 = ||x||/16;  rs = 16/||x||
                n16 = ep.tile([128, 1], F32, tag="n16", bufs=3)
                nc.scalar.activation(out=n16, in_=ss, func=AF.Sqrt, bias=0.0,
                                     scale=1.0 / (SCALE * SCALE))
                rs = ep.tile([128, 1], F32, tag="rs", bufs=3)
                nc.vector.reciprocal(out=rs, in_=n16)
                xq = ep.tile([128, H], BF16, tag="xq", bufs=3)
                nc.vector.tensor_scalar(xq, x, rs, None, op0=OP.mult)
                tp = trps.tile([128, H], BF16, tag="tr")
                for kt in range(KT):
                    nc.tensor.transpose(tp[:, kt * 128:(kt + 1) * 128],
                                        xq[:, kt * 128:(kt + 1) * 128],
                                        identb)
                pend.append((tp, bt))
                if len(pend) > LAG:
                    drain()
            while pend and flush:
                drain()
            return pend if not flush else None

        prep(emb, embt, BT, True)
        prep(emb_s, stript, MT, True)

    # ---- Phase C: fp8 DoubleRow sim matmul + threshold + U ----
    if "C" in which:
     with tc.tile_pool(name=f"{r}simps", bufs=3, space="PSUM") as sps, \
         tc.tile_pool(name=f"{r}ups", bufs=2, space="PSUM") as ups, \
         tc.tile_pool(name=f"{r}utps", bufs=2, space="PSUM") as utps, \
         tc.tile_pool(name=f"{r}mkp", bufs=3) as mkp, \
         tc.tile_pool(name=f"{r}stg", bufs=2) as stg:
        # Software-pipelined: the U matmul for mask m is emitted after the
        # sim matmuls of m+1 (so the PE never waits on the DVE threshold of
        # the tile it just produced), and the U^T staging of column tile n
        # is emitted inside the m-loop of tile n+1.
        def stage(u_, n_):
            ust = stg.tile([E + 1, 512], F32, tag="ust")
            nc.scalar.copy(out=ust, in_=u_)
            for c in range(4):
                jt = n_ * 4 + c
                tp = utps.tile([128, E + 1], F32, tag="ut")
                nc.tensor.matmul(out=tp,
                                 lhsT=ust[:, c * 128:(c + 1) * 128],
                                 rhs=identf[:E + 1, :E + 1],
                                 start=True, stop=True)
                if c % 2 == 0:
                    nc.vector.tensor_copy(out=Ut_all[:, jt, :], in_=tp)
                else:
                    nc.scalar.copy(out=Ut_all[:, jt, :], in_=tp)

        prev = None
        for n in range(NT):
            u = ups.tile([E + 1, 512], F32, tag="u")
            msks = [None] * MT
            for m in range(MT):
                sim = sps.tile([128, 512], F32, tag="sim")
                for k2 in range(KT2):
                    nc.tensor.matmul(
                        out=sim,
                        lhsT=stript[:, 2 * k2:2 * k2 + 2,
                                    m * 128:(m + 1) * 128],
                        rhs=embt[:, 2 * k2:2 * k2 + 2,
                                 n * 512:(n + 1) * 512],
                        start=(k2 == 0), stop=(k2 == KT2 - 1),
                        perf_mode=DR)
                msk = mkp.tile([128, 512], BF16, tag="mask", bufs=4)
                nc.vector.tensor_scalar(msk, sim, THRESH, None, op0=OP.is_gt)
                msks[m] = msk
                if m >= 1:
                    nc.tensor.matmul(out=u, lhsT=Lpad[:, m - 1, :],
                                     rhs=msks[m - 1], start=(m == 1),
                                     stop=False)
                if m == 2 and prev is not None:
                    stage(*prev)
            nc.tensor.matmul(out=u, lhsT=Lpad[:, MT - 1, :],
                             rhs=msks[MT - 1], start=False, stop=True)
            prev = (u, n)
        stage(*prev)

    # ---- Phase D: final reduction to (masked_sum, count) ----
    if "D" in which:
     with tc.tile_pool(name=f"{r}fin", bufs=1) as fin, \
         tc.tile_pool(name=f"{r}fps", bufs=1, space="PSUM") as fps:
        scr = fin.tile([128, BT, E + 1], F32)
        nc.vector.tensor_tensor(out=scr, in0=W17, in1=Ut_all, op=OP.mult)
        accs = fin.tile([128, 2], F32)
        nc.vector.reduce_sum(out=accs[:, 0:1], in_=scr, axis=AXY)
        nc.vector.reduce_sum(out=accs[:, 1:2], in_=Ut_all[:, :, E:E + 1],
                             axis=AXY)
        res = fps.tile([1, 2], F32)
        nc.tensor.matmul(out=res, lhsT=ones, rhs=accs, start=True, stop=True)
        out_sb = fin.tile([1, 2], F32)
        nc.scalar.copy(out=out_sb, in_=res)
        nc.sync.dma_start(out=out_dram, in_=out_sb)


def build_bass(reps=1, loop_iters=None, phases="ABCD"):
    nc = bacc.Bacc("TRN2", target_bir_lowering=False, debug=False)
    emb = nc.dram_tensor("emb", [B, H], F32, kind="ExternalInput").ap()
    emb_s = nc.dram_tensor("emb_strip", [STRIP, H], F32,
                           kind="ExternalInput").ap()
    rp = nc.dram_tensor("rp", [B, E], F32, kind="ExternalInput").ap()
    rp_s = nc.dram_tensor("rp_strip", [STRIP, E], F32,
                          kind="ExternalInput").ap()
    out = nc.dram_tensor("out", [1, 2], F32, kind="ExternalOutput").ap()
    with tile.TileContext(nc) as tc:
        _kernel(tc, emb, emb_s, rp, rp_s, out, reps=reps,
                loop_iters=loop_iters, phases=phases)
    nc.compile()
    return nc


_NC_CACHE = None


def make_in_maps(rp, emb):
    in_maps = []
    for d in range(NCORES):
        in_maps.append({
            "emb": emb,
            "emb_strip": np.ascontiguousarray(emb[d * STRIP:(d + 1) * STRIP]),
            "rp": rp,
            "rp_strip": np.ascontiguousarray(rp[d * STRIP:(d + 1) * STRIP]),
        })
    return in_maps


def kernel(routing_probs: np.ndarray, input_embeddings: np.ndarray,
           **_unused) -> np.ndarray:
    global _NC_CACHE
    if _NC_CACHE is None:
        _NC_CACHE = build_bass()
    nc = _NC_CACHE
    rp = np.ascontiguousarray(routing_probs, dtype=np.float32)
    emb = np.ascontiguousarray(input_embeddings, dtype=np.float32)
    in_maps = make_in_maps(rp, emb)
    res = run_bass_kernel_spmd(nc, in_maps, core_ids=list(range(NCORES)))
    vals = np.array([r["out"].reshape(2) for r in res.results],
                    dtype=np.float64)
    total = vals[:, 0].sum()
    cnt = vals[:, 1].sum() - B  # drop the diagonal pairs (kl there is 0)
    if cnt > 0:
        loss = np.float32(total) / np.float32(max(cnt, 1.0))
    else:
        loss = 0.0
    return np.array(WEIGHT * loss, dtype=np.float32)


# revision 25
# speedup vs baseline: 6.9839x; 1.4157x over previous
"""Trainium2 Bass kernel for the batch ConsistencyLoss (masked pairwise KL).

Math (reference):
    emb = x / ||x||;  sim = emb @ emb.T;  mask = (sim > 0.8) & ~eye
    L = log_softmax(routing);  P = exp(L);  ne[j] = sum_k P[j,k] L[j,k]
    kl[i,j] = ne[j] - (L @ P.T)[i,j]
    loss = sum(mask * kl) / count(mask)

Device algorithm (per core, row strip S of 1024 rows):
  * Rows are normalized to ||u|| = 16 in the natural [row, H] layout
    (ACT Square+accum for ||x||^2, DVE multiply), quantized to fp8e4 at
    the PSUM->SBUF copy-out of the PE transpose.  The similarity test is
    then a compare against the CONSTANT 0.8*16*16 = 204.8.
  * sim is computed TRANSPOSED, one 128-row block of j at a time:
        simT[j, i] = q(u_j) . q(u_i),   j in block jt, i in strip S
    with fp8 DoubleRow matmuls (contraction 256/instruction, stationary
    = embt block, moving = strip columns).  maskT = simT > 204.8.
  * Masked-KL sum, fully on the transposed side:
        V[k, i]  = sum_j W18[j, k] * maskT[j, i]   (PSUM-accumulated
                   over ALL jt blocks; W18 = [-P | ne | 1], 18 cols)
        masked_sum = sum_{k,i} LTpad[k, i] * V[k, i]
        count      = sum_i V[17, i]
    with LTpad = [L_S^T ; 1 ; 0] (18 x S).  No U staging or transposes.
  * The jt sim block is emitted right after embedding tile jt is
    transposed, so DMA / normalize / transpose / sim / V form one
    software-pipelined loop that keeps the PE continuously busy.
  * Diagonal pairs have kl == 0 exactly, so they stay in the mask and
    the host subtracts B from the pair count.
"""

import numpy as np

import concourse.bacc as bacc
import concourse.tile as tile
from concourse import mybir
from concourse.bass_utils import run_bass_kernel_spmd
from concourse.masks import make_identity

B, E, H = 8192, 16, 1024
NCORES = 8
STRIP = B // NCORES  # 1024 rows per core
MT = STRIP // 128    # 8 row chunks per strip
KT = H // 128        # 8 contraction tiles
KT2 = KT // 2        # 4 DoubleRow contraction pairs
BT = B // 128        # 64 batch tiles (also the jt sim blocks)
W = E + 2            # [-P | ne | 1] stationary width
SIM_THRESHOLD = 0.8
SCALE = 16.0         # rows normalized to this L2 norm before fp8 quant
THRESH = SIM_THRESHOLD * SCALE * SCALE  # 204.8 in raw-dot units
WEIGHT = 1.0
F32 = mybir.dt.float32
BF16 = mybir.dt.bfloat16
F8 = mybir.dt.float8e4
AX = mybir.AxisListType.X
OP = mybir.AluOpType
AF = mybir.ActivationFunctionType
DR = mybir.MatmulPerfMode.DoubleRow


def _kernel(tc, emb, emb_s, rp, rp_s, out_dram, reps=1, loop_iters=None,
            phases="ABCD"):
    nc = tc.nc
    with tc.tile_pool(name="persist", bufs=1) as persist:
        embt = persist.tile([128, KT, B], F8)          # q(u)^T [h%128,kt,b]
        stript = persist.tile([128, KT, STRIP], F8)    # strip columns
        W18 = persist.tile([128, BT, W], BF16)         # [-P | ne | 1]
        LTpad = persist.tile([W, STRIP], F32)          # [L^T ; 1 ; 0]
        identf = persist.tile([128, 128], F32)
        identb = persist.tile([128, 128], BF16)
        ones18 = persist.tile([W, 1], F32)
        make_identity(nc, identf)
        make_identity(nc, identb)
        nc.vector.memset(ones18, 1.0)
        if "B" not in phases and "C" in phases:
            # timing variants only: C reads embt/stript without B writing
            for kt in range(KT):
                nc.gpsimd.memset(embt[:, kt, :], 0.5)
                nc.gpsimd.memset(stript[:, kt, :], 0.5)

        args = (tc, nc, emb, emb_s, rp, rp_s, out_dram, embt, stript,
                W18, LTpad, identf, identb, ones18)
        if loop_iters is not None:
            with tc.For_i(0, loop_iters, 1):
                _phases(*args, "", phases)
            return
        for rep in range(reps):
            _phases(*args, f"r{rep}_" if reps > 1 else "", phases)


def _phases(tc, nc, emb, emb_s, rp, rp_s, out_dram, embt, stript,
            W18, LTpad, identf, identb, ones18, r, which="ABCD"):
    # ---- Phase A: softmax stats (full batch -P/ne/1; strip L^T) ----
    # All Exp ops batch under one ACT table; the 72 Ln calls collapse into
    # ONE Ln over the collected sums (ACT table loads: ~2 instead of ~99).
    TT = BT + MT
    if "A" in which:
     with tc.tile_pool(name=f"{r}smx", bufs=1) as smx, \
          tc.tile_pool(name=f"{r}ltp", bufs=2, space="PSUM") as ltps:
        rp_sb = smx.tile([128, BT, E], F32, tag="rp_sb")
        rps_sb = smx.tile([128, MT, E], F32, tag="rps_sb")
        nc.sync.dma_start(
            out=rp_sb, in_=rp.rearrange("(bt p) e -> p bt e", p=128))
        nc.sync.dma_start(
            out=rps_sb, in_=rp_s.rearrange("(mt p) e -> p mt e", p=128))
        e_all = smx.tile([128, TT, E], F32, tag="e_all")
        s_all = smx.tile([128, TT], F32, tag="s_all")
        nm_all = smx.tile([128, TT], F32, tag="nm_all")
        logs_all = smx.tile([128, TT], F32, tag="logs_all")
        rs_all = smx.tile([128, TT], F32, tag="rs_all")

        def logits(t):
            return rp_sb[:, t, :] if t < BT else rps_sb[:, t - BT, :]

        for t in range(TT):
            nc.vector.reduce_max(out=nm_all[:, t:t + 1], in_=logits(t),
                                 axis=AX, negate=True)
            nc.scalar.activation(out=e_all[:, t, :], in_=logits(t),
                                 func=AF.Exp, bias=nm_all[:, t:t + 1],
                                 scale=1.0, accum_out=s_all[:, t:t + 1])
        nc.scalar.activation(out=logs_all, in_=s_all, func=AF.Ln)
        nc.vector.reciprocal(out=rs_all, in_=s_all)
        nc.vector.memset(W18[:, :, E + 1], 1.0)
        for bt in range(BT):
            nc.vector.tensor_scalar(W18[:, bt, 0:E], e_all[:, bt, :],
                                    rs_all[:, bt:bt + 1], -1.0,
                                    op0=OP.mult, op1=OP.mult)
            L = smx.tile([128, E], F32, tag="Lf", bufs=2)
            nc.vector.tensor_scalar(L, rp_sb[:, bt, :], nm_all[:, bt:bt + 1],
                                    logs_all[:, bt:bt + 1], op0=OP.add,
                                    op1=OP.subtract)
            scr = smx.tile([128, E], F32, tag="nescr", bufs=2)
            nc.vector.tensor_tensor(out=scr, in0=W18[:, bt, 0:E], in1=L,
                                    op=OP.mult)
            with nc.allow_low_precision(reason="ne reduce over 16 vals"):
                nc.vector.reduce_sum(out=W18[:, bt, E:E + 1], in_=scr,
                                     axis=AX, negate=True)
        # LTpad rows 0..15 = L^T, row 16 = 1 (via transpose of [L | 1]),
        # row 17 = 0 (whole-tile memset; partition slices must start at 0)
        nc.vector.memset(LTpad, 0.0)
        for ms in range(MT):
            t = BT + ms
            Lm = smx.tile([128, E + 1], F32, tag="Lm", bufs=3)
            nc.vector.memset(Lm[:, E:E + 1], 1.0)
            nc.vector.tensor_scalar(Lm[:, 0:E], rps_sb[:, ms, :],
                                    nm_all[:, t:t + 1], logs_all[:, t:t + 1],
                                    op0=OP.add, op1=OP.subtract)
            lt = ltps.tile([E + 1, 128], F32, tag="lt")
            nc.tensor.matmul(out=lt, lhsT=Lm, rhs=identf,
                             start=True, stop=True)
            nc.scalar.copy(out=LTpad[0:E + 1, ms * 128:(ms + 1) * 128],
                           in_=lt)

    # ---- Phase B+C: merged pipeline ----
    #   prep(bt):  DMA -> norm^2 (ACT) -> 16/||x|| (DVE) -> normalize (DVE)
    #              -> PE transpose -> fp8 copy-out (ACT/DVE alternating)
    #   simblk(jt): 8 DoubleRow matmuls (2 column halves) -> maskT (DVE)
    #              -> V matmul (accumulated over all jt)
    # simblk(jt) is emitted two prep steps behind, V one jt behind, so no
    # in-order engine stream ever waits on same-step work.
    do_b = "B" in which
    do_c = "C" in which
    if do_b or do_c:
     with tc.tile_pool(name=f"{r}embp", bufs=3) as ep, \
          tc.tile_pool(name=f"{r}trps", bufs=2, space="PSUM") as trps, \
          tc.tile_pool(name=f"{r}simps", bufs=3, space="PSUM") as sps, \
          tc.tile_pool(name=f"{r}vps", bufs=1, space="PSUM") as vps, \
          tc.tile_pool(name=f"{r}mkp", bufs=6) as mkp:
        V = vps.tile([W, STRIP], F32, name="V") if do_c else None
        tpend = []   # pending transpose copy-outs  (tp, dst, bt)
        vpend = []   # pending V matmuls            (jt, [msk0, msk1])

        def drain_tp():
            tp_, dst_, bt_ = tpend.pop(0)
            eng = (nc.scalar.copy if bt_ % 2 == 0 else nc.vector.tensor_copy)
            eng(out=dst_[:, :, bt_ * 128:(bt_ + 1) * 128],
                in_=tp_.rearrange("p (k c) -> p k c", k=KT))

        def prep(src_ap, dst_tile, bt):
            x = ep.tile([128, H], F32, tag="ex", bufs=4)
            nc.sync.dma_start(out=x, in_=src_ap[bt * 128:(bt + 1) * 128, :])
            scr = ep.tile([128, H], BF16, tag="sqscr", bufs=2)
            ss = ep.tile([128, 1], F32, tag="ss", bufs=3)
            nc.scalar.activation(out=scr, in_=x, func=AF.Square,
                                 accum_out=ss)
            # n16 = ||x||/16;  rs = 16/||x||
            n16 = ep.tile([128, 1], F32, tag="n16", bufs=3)
            nc.scalar.activation(out=n16, in_=ss, func=AF.Sqrt, bias=0.0,
                                 scale=1.0 / (SCALE * SCALE))
            rs = ep.tile([128, 1], F32, tag="rs", bufs=3)
            nc.vector.reciprocal(out=rs, in_=n16)
            xq = ep.tile([128, H], BF16, tag="xq", bufs=3)
            nc.vector.tensor_scalar(xq, x, rs, None, op0=OP.mult)
            if tpend:
                drain_tp()
            tp = trps.tile([128, H], BF16, tag="tr")
            for kt in range(KT):
                nc.tensor.transpose(tp[:, kt * 128:(kt + 1) * 128],
                                    xq[:, kt * 128:(kt + 1) * 128],
                                    identb)
            tpend.append((tp, dst_tile, bt))

        def drain_v(stop):
            jt_, msks_ = vpend.pop(0)
            for hh in range(2):
                nc.tensor.matmul(out=V[:, hh * 512:(hh + 1) * 512],
                                 lhsT=W18[:, jt_, :], rhs=msks_[hh],
                                 start=(jt_ == 0), stop=stop)

        def simblk(jt):
            msks = []
            for hh in range(2):
                simT = sps.tile([128, 512], F32, tag="simT")
                for k2 in range(KT2):
                    nc.tensor.matmul(
                        out=simT,
                        lhsT=embt[:, 2 * k2:2 * k2 + 2,
                                  jt * 128:(jt + 1) * 128],
                        rhs=stript[:, 2 * k2:2 * k2 + 2,
                                   hh * 512:(hh + 1) * 512],
                        start=(k2 == 0), stop=(k2 == KT2 - 1),
                        perf_mode=DR)
                msk = mkp.tile([128, 512], BF16, tag="mask")
                nc.vector.tensor_scalar(msk, simT, THRESH, None, op0=OP.is_gt)
                msks.append(msk)
            if vpend:
                drain_v(False)
            vpend.append((jt, msks))

        if do_b:
            for ms in range(MT):           # strip prologue
                prep(emb_s, stript, ms)
        CLAG = 2
        for bt in range(BT):
            if do_b:
                prep(emb, embt, bt)
            if do_c and bt >= CLAG:
                simblk(bt - CLAG)
        for jt in range(BT - CLAG, BT) if do_c else []:
            simblk(jt)
        while tpend:
            drain_tp()
        if do_c:
            drain_v(True)

            # ---- readout: masked_sum and count from V ----
            with tc.tile_pool(name=f"{r}fin", bufs=1) as fin, \
                 tc.tile_pool(name=f"{r}fps", bufs=1, space="PSUM") as fps:
                Vs = fin.tile([W, STRIP], F32)
                nc.scalar.copy(out=Vs, in_=V)
                scr = fin.tile([W, STRIP], F32)
                nc.vector.tensor_tensor(out=scr, in0=Vs, in1=LTpad,
                                        op=OP.mult)
                # accs col0 = rowsum(LTpad*V) (-> masked_sum), col1 =
                # rowsum(V); select row 17 of col1 (count) by multiplying
                # with [1 | e17] built from the identity's column 17.
                accs = fin.tile([W, 2], F32)
                nc.vector.reduce_sum(out=accs[:, 0:1], in_=scr, axis=AX)
                nc.vector.reduce_sum(out=accs[:, 1:2], in_=Vs, axis=AX)
                sel = fin.tile([W, 2], F32)
                nc.vector.tensor_copy(out=sel[:, 0:1], in_=ones18)
                nc.vector.tensor_copy(out=sel[:, 1:2],
                                      in_=identf[0:W, W - 1:W])
                msel = fin.tile([W, 2], F32)
                nc.vector.tensor_tensor(out=msel, in0=accs, in1=sel,
                                        op=OP.mult)
                res = fps.tile([1, 2], F32)
                nc.tensor.matmul(out=res, lhsT=ones18, rhs=msel,
                                 start=True, stop=True)
                out_sb = fin.tile([1, 2], F32)
                nc.scalar.copy(out=out_sb, in_=res)
                nc.sync.dma_start(out=out_dram, in_=out_sb)


def build_bass(reps=1, loop_iters=None, phases="ABCD"):
    nc = bacc.Bacc("TRN2", target_bir_lowering=False, debug=False)
    emb = nc.dram_tensor("emb", [B, H], F32, kind="ExternalInput").ap()
    emb_s = nc.dram_tensor("emb_strip", [STRIP, H], F32,
                           kind="ExternalInput").ap()
    rp = nc.dram_tensor("rp", [B, E], F32, kind="ExternalInput").ap()
    rp_s = nc.dram_tensor("rp_strip", [STRIP, E], F32,
                          kind="ExternalInput").ap()
    out = nc.dram_tensor("out", [1, 2], F32, kind="ExternalOutput").ap()
    with tile.TileContext(nc) as tc:
        _kernel(tc, emb, emb_s, rp, rp_s, out, reps=reps,
                loop_iters=loop_iters, phases=phases)
    nc.compile()
    return nc


_NC_CACHE = None


def make_in_maps(rp, emb):
    in_maps = []
    for d in range(NCORES):
        in_maps.append({
            "emb": emb,
            "emb_strip": np.ascontiguousarray(emb[d * STRIP:(d + 1) * STRIP]),
            "rp": rp,
            "rp_strip": np.ascontiguousarray(rp[d * STRIP:(d + 1) * STRIP]),
        })
    return in_maps


def kernel(routing_probs: np.ndarray, input_embeddings: np.ndarray,
           **_unused) -> np.ndarray:
    global _NC_CACHE
    if _NC_CACHE is None:
        _NC_CACHE = build_bass()
    nc = _NC_CACHE
    rp = np.ascontiguousarray(routing_probs, dtype=np.float32)
    emb = np.ascontiguousarray(input_embeddings, dtype=np.float32)
    in_maps = make_in_maps(rp, emb)
    res = run_bass_kernel_spmd(nc, in_maps, core_ids=list(range(NCORES)))
    vals = np.array([r["out"].reshape(2) for r in res.results],
                    dtype=np.float64)
    total = vals[:, 0].sum()
    cnt = vals[:, 1].sum() - B  # drop the diagonal pairs (kl there is 0)
    if cnt > 0:
        loss = np.float32(total) / np.float32(max(cnt, 1.0))
    else:
        loss = 0.0
    return np.array(WEIGHT * loss, dtype=np.float32)
